# revision 1
# baseline (speedup 1.0000x reference)
"""MAGNO encoder kernel distributed across 8 Trainium2 NeuronCores.

Sharding: device d in [0,8) handles batch b = d//4 and latent-row quarter
q = d%4 (rows [4096q, 4096(q+1))). row_idx is sorted, so each (batch,
scale, quarter)'s edges are a contiguous range — the host finds the
boundaries, pads each range to a common cap, and ships local row ids.
Every edge for a given latent row lands on exactly one device, so each
device's segment sums/counts are already global: it applies the mean and
softmax scale-weighting on device and returns its final disjoint
[4096, COUT] output slice. The host just concatenates the 8 slices.

Two device stages (separate pmaps): gather+MLP, then segment-sum+finish.
"""

import numpy as np
import jax
import jax.numpy as jnp
from functools import partial

B, N, M, S, E = 2, 100000, 16384, 3, 262144
CD, CIN, COUT, HID = 2, 3, 32, 64
NDEV = 8
MQ = M // 4  # latent rows per device


def _kernel_mlp(a, W1, b1, W2, b2, W3, b3):
    h = jax.nn.gelu(a @ W1.T + b1)
    h = jax.nn.gelu(h @ W2.T + b2)
    return h @ W3.T + b3


@partial(jax.pmap, axis_name="d",
         in_axes=(0, 0, 0, 0, None, None, None, None, None, None, None,
                  None, None))
def _stage_a(xb, pnb_raw, nbrs, rows_g, lat,
             W_lift, b_lift, W1, b1, W2, b2, W3, b3):
    pn = pnb_raw @ W_lift.T + b_lift                       # [N, COUT]
    ks = []
    for i in range(S):
        nbr, row = nbrs[i], rows_g[i]
        a = jnp.concatenate([xb[nbr], lat[row]], axis=-1)  # [ECAP, 2CD]
        k = _kernel_mlp(a, W1, b1, W2, b2, W3, b3)         # [ECAP, COUT]
        ks.append(k * pn[nbr])
    return jnp.stack(ks)                                   # [S, ECAP, COUT]


@partial(jax.pmap, axis_name="d", in_axes=(0, 0, 0))
def _stage_b(ks, rows_l, wcnt_q):
    # rows_l in [0, MQ] (MQ = padding sentinel)
    # wcnt_q: [S, MQ] = softmax_weight / max(count, 1), host-precomputed,
    # so the segment mean + scale weighting collapse to one multiply.
    acc = jnp.zeros((MQ, COUT), jnp.float32)
    for i in range(S):
        s = jax.ops.segment_sum(ks[i], rows_l[i], num_segments=MQ + 1,
                                indices_are_sorted=True)[:MQ]
        acc = acc + s * wcnt_q[i][:, None]
    return acc                                             # [MQ, COUT]


def _softmax_weights(lat, Ws1, bs1, Ws2, bs2):
    h = np.maximum(lat @ Ws1.T + bs1, 0.0) @ Ws2.T + bs2   # [M, S]
    h -= h.max(axis=-1, keepdims=True)
    e = np.exp(h)
    return e / e.sum(axis=-1, keepdims=True)               # [M, S]


def _numpy_fallback(x_coord, pndata, lat, nbr, row, W_lift, b_lift,
                    W1, b1, W2, b2, W3, b3, sw):
    def gelu(x):
        return 0.5 * x * (1.0 + np.tanh(np.sqrt(2 / np.pi) * (x + 0.044715 * x ** 3)))
    out = np.zeros((B, M, COUT), np.float32)
    for b in range(B):
        pn = pndata[b] @ W_lift.T + b_lift
        for s in range(S):
            nb, rw = nbr[b, s], row[b, s]
            a = np.concatenate([x_coord[b][nb], lat[rw]], axis=-1)
            h = gelu(a @ W1.T + b1)
            h = gelu(h @ W2.T + b2)
            k = (h @ W3.T + b3) * pn[nb]
            sums = np.zeros((M, COUT), np.float32)
            cnts = np.zeros((M,), np.float32)
            np.add.at(sums, rw, k)
            np.add.at(cnts, rw, 1.0)
            out[b] += (sums / np.maximum(cnts, 1.0)[:, None]) * sw[:, s][:, None]
    return out


def kernel(x_coord, pndata, latent_tokens_coord, nbr_idx, row_idx,
           W_lift, b_lift, W1, b1, W2, b2, W3, b3, Ws1, bs1, Ws2, bs2):
    x_coord = np.asarray(x_coord, dtype=np.float32)
    pndata = np.asarray(pndata, dtype=np.float32)
    lat = np.asarray(latent_tokens_coord, dtype=np.float32)
    nbr = np.asarray(nbr_idx).astype(np.int32)
    row = np.asarray(row_idx).astype(np.int32)
    f32 = lambda a: np.asarray(a, dtype=np.float32)
    Wl, bl = f32(W_lift), f32(b_lift)
    W1f, b1f, W2f, b2f, W3f, b3f = map(f32, (W1, b1, W2, b2, W3, b3))
    sw = _softmax_weights(lat, f32(Ws1), f32(bs1), f32(Ws2), f32(bs2))

    # row-quarter boundaries per (b, s): rows are sorted along E
    bounds = np.empty((B, S, 5), np.int64)
    for b in range(B):
        for s in range(S):
            bounds[b, s] = np.searchsorted(row[b, s], [0, MQ, 2 * MQ, 3 * MQ, M])
    caps = bounds[:, :, 1:] - bounds[:, :, :-1]            # chunk lengths
    ecap = int(-(-int(caps.max()) // 1024) * 1024)         # pad to 1K multiple

    xb = np.empty((NDEV, N, CD), np.float32)
    pnb = np.empty((NDEV, N, CIN), np.float32)
    nbrs = np.zeros((NDEV, S, ecap), np.int32)
    rows_g = np.zeros((NDEV, S, ecap), np.int32)           # global (for lat)
    rows_l = np.full((NDEV, S, ecap), MQ, np.int32)        # local (for scatter)
    wcnt_q = np.empty((NDEV, S, MQ), np.float32)
    for d in range(NDEV):
        b, q = d // 4, d % 4
        xb[d] = x_coord[b]
        pnb[d] = pndata[b]
        for s in range(S):
            lo, hi = bounds[b, s, q], bounds[b, s, q + 1]
            n = hi - lo
            nbrs[d, s, :n] = nbr[b, s, lo:hi]
            rows_g[d, s, :n] = row[b, s, lo:hi]
            rl = row[b, s, lo:hi] - q * MQ
            rows_l[d, s, :n] = rl
            cnt = np.bincount(rl, minlength=MQ).astype(np.float32)
            wcnt_q[d, s] = sw[q * MQ:(q + 1) * MQ, s] / np.maximum(cnt, 1.0)

    try:
        ks = _stage_a(xb, pnb, nbrs, rows_g, lat, Wl, bl,
                      W1f, b1f, W2f, b2f, W3f, b3f)
        out_d = _stage_b(ks, rows_l, wcnt_q)
        out_q = np.asarray(jax.device_get(out_d))          # [8, MQ, COUT]
        out = np.empty((B, M, COUT), np.float32)
        for d in range(NDEV):
            b, q = d // 4, d % 4
            out[b, q * MQ:(q + 1) * MQ] = out_q[d]
        return out
    except Exception:
        return _numpy_fallback(x_coord, pndata, lat, nbr, row, Wl, bl,
                               W1f, b1f, W2f, b2f, W3f, b3f, sw)

